# revision 13
# baseline (speedup 1.0000x reference)
"""Multi-head attention (B=2, S=2048, D=1024, H=16, dk=dv=64) with ALiBi bias,
returning (output, softmax weights), on 8 Trainium2 NeuronCores.

Sharding: batch x head-group (2 x 4). Each core computes 4 heads of one batch:
QKV projections column-sharded, scores/softmax/context local, Wo row-sharded
with the cross-head partial sum reduced on the host.

Numerics: matmuls run in float32r (~13-bit mantissa, full PE rate for moving
dim >= 256). The ALiBi bias is folded into the scores matmul via an augmented
contraction row holding 8*(bias - C_h) with C_h = slope_h*2047, so the
(large) bias values only carry f32r rounding error where the softmax weight
is negligible. Softmax needs no max-reduction: exponents are <= ~6 by
construction. exp runs on ScalarE with per-partition bias + accumulated row
sums; normalization and PSUM evacuation on VectorE.
"""

import math

import numpy as np

import concourse.mybir as mybir
import concourse.tile as tile
from concourse import bacc
from concourse import bass_utils
from concourse.masks import make_identity

B, S, DM, H, DK, DV = 2, 2048, 1024, 16, 64, 64
HPC = 4  # heads per core
NCORES = 8
FP32 = mybir.dt.float32
F32R = mybir.dt.float32r
EXPF = mybir.ActivationFunctionType.Exp
NSB = S // 128  # 16 s-blocks
NDB = DM // 128  # 8 dmodel-blocks
CH = 512
NCH = S // CH  # 4 chunks


def _alibi_slopes(n_heads):
    def pow2_slopes(n):
        start = 2.0 ** (-(2.0 ** (-(math.log2(n) - 3))))
        return [start * (start**i) for i in range(n)]

    if math.log2(n_heads).is_integer():
        return pow2_slopes(n_heads)
    closest = 2 ** math.floor(math.log2(n_heads))
    return pow2_slopes(closest) + _alibi_slopes(2 * closest)[0::2][: n_heads - closest]


def build_module():
    nc = bacc.Bacc("TRN2", target_bir_lowering=False, debug=False, num_devices=NCORES)

    q_d = nc.dram_tensor("q", [S, DM], FP32, kind="ExternalInput").ap()
    k_d = nc.dram_tensor("k", [S, DM], FP32, kind="ExternalInput").ap()
    v_d = nc.dram_tensor("v", [S, DM], FP32, kind="ExternalInput").ap()
    wq_d = nc.dram_tensor("wq", [DM, HPC * DK], FP32, kind="ExternalInput").ap()
    wk_d = nc.dram_tensor("wk", [DM, HPC * DK], FP32, kind="ExternalInput").ap()
    wv_d = nc.dram_tensor("wv", [DM, HPC * DV], FP32, kind="ExternalInput").ap()
    wo_d = nc.dram_tensor("wo", [HPC * DV, DM], FP32, kind="ExternalInput").ap()
    alibi8_d = nc.dram_tensor("alibi8", [HPC + 1, S], FP32, kind="ExternalInput").ap()
    biascol_d = nc.dram_tensor("biascol", [128, HPC * NSB], FP32, kind="ExternalInput").ap()
    wout_d = nc.dram_tensor("wout", [HPC, S, S], FP32, kind="ExternalOutput").ap()
    oout_d = nc.dram_tensor("oout", [S, DM], FP32, kind="ExternalOutput").ap()

    with tile.TileContext(nc) as tc:
        with (
            tc.tile_pool(name="persist", bufs=1) as persist,
            tc.tile_pool(name="strip", bufs=7) as stripp,
            tc.tile_pool(name="aug", bufs=4) as augp,
            tc.tile_pool(name="inp", bufs=3) as inpp,
            tc.tile_pool(name="obuf", bufs=3) as obufp,
            tc.tile_pool(name="bc", bufs=2) as bcp,
            tc.tile_pool(name="small", bufs=8) as smallp,
            tc.tile_pool(name="mm", bufs=4, space="PSUM") as mmp,
            tc.tile_pool(name="ctxp", bufs=1, space="PSUM") as ctxpp,
        ):
            ident = persist.tile([128, 128], FP32, tag="ident")
            make_identity(nc, ident)
            biascol_sb = persist.tile([128, HPC * NSB], FP32, tag="biascol")
            nc.sync.dma_start(biascol_sb, biascol_d)

            v_sb = persist.tile([128, NSB * HPC * DV], F32R, tag="vsb")
            gk = {}  # (tensor_idx, grp) -> [128, S] f32r projection outputs
            ctxT = [
                persist.tile([128, S], F32R, tag=f"ctxT{g}", name=f"ctxT{g}")
                for g in range(2)
            ]

            # --- weight shards -> SBUF, rounded to f32r ---
            def load_w(wdram, nblk):
                st = stripp.tile([128, nblk * wdram.shape[1]], FP32, tag="strip")
                nc.sync.dma_start(
                    st.rearrange("p (nb m) -> p nb m", nb=nblk),
                    wdram.rearrange("(nb p) m -> p nb m", p=128),
                )
                wsb = stripp.tile([128, nblk * wdram.shape[1]], F32R, tag="strip")
                nc.vector.tensor_copy(wsb, st)
                return wsb

            wq_sb = load_w(wq_d, NDB)  # [128, 8*256]
            wk_sb = load_w(wk_d, NDB)
            wv_sb = load_w(wv_d, NDB)
            wo_sb = load_w(wo_d, 2)  # [128, 2*1024]

            # --- phase 1: transpose inputs, project ---
            for t_idx, (src_d, wsb) in enumerate(
                [(q_d, wq_sb), (k_d, wk_sb), (v_d, wv_sb)]
            ):
                is_v = t_idx == 2
                if not is_v:
                    for g in range(2):
                        gk[(t_idx, g)] = persist.tile(
                            [128, S], F32R, tag=f"gk{t_idx}{g}", name=f"gk{t_idx}{g}"
                        )
                for chunk in range(NCH):
                    colA = stripp.tile([128, 4 * 512], F32R, tag="strip")
                    colB = stripp.tile([128, 4 * 512], F32R, tag="strip")

                    def col(db, lo, sz):
                        t = colA if db < 4 else colB
                        return t[:, (db % 4) * 512 + lo : (db % 4) * 512 + lo + sz]

                    for j in range(4):
                        sblk = chunk * 4 + j
                        it = inpp.tile([128, DM], FP32, tag="inp")
                        nc.sync.dma_start(it, src_d[sblk * 128 : (sblk + 1) * 128, :])
                        for db in range(NDB):
                            pt = mmp.tile([128, 128], FP32, tag="mm")
                            nc.tensor.transpose(
                                pt, it[:, db * 128 : (db + 1) * 128], ident
                            )
                            nc.vector.tensor_copy(col(db, j * 128, 128), pt)
                    if not is_v:
                        for g in range(2):
                            ps = mmp.tile([128, 512], FP32, tag="mm")
                            for db in range(NDB):
                                nc.tensor.matmul(
                                    ps,
                                    wsb[:, db * 256 + g * 128 : db * 256 + (g + 1) * 128],
                                    col(db, 0, 512),
                                    start=(db == 0),
                                    stop=(db == NDB - 1),
                                )
                            nc.vector.tensor_copy(
                                gk[(t_idx, g)][:, chunk * 512 : (chunk + 1) * 512], ps
                            )
                    else:
                        for j in range(4):
                            sblk = chunk * 4 + j
                            ps = mmp.tile([128, HPC * DV], FP32, tag="mm")
                            for db in range(NDB):
                                nc.tensor.matmul(
                                    ps,
                                    col(db, j * 128, 128),
                                    wsb[:, db * 256 : (db + 1) * 256],
                                    start=(db == 0),
                                    stop=(db == NDB - 1),
                                )
                            nc.vector.tensor_copy(
                                v_sb[:, sblk * 256 : (sblk + 1) * 256], ps
                            )

            # --- phase 2: per-head attention ---
            for h in range(HPC):
                g, r = h // 2, (h % 2) * 64
                qa = augp.tile([65, S], F32R, tag="aug")
                ka = augp.tile([65, S], F32R, tag="aug")
                nc.sync.dma_start(qa[0:64, :], gk[(0, g)][r : r + 64, :])
                nc.sync.dma_start(qa[64:65, :], alibi8_d[HPC : HPC + 1, :].bitcast(F32R))
                nc.sync.dma_start(ka[0:64, :], gk[(1, g)][r : r + 64, :])
                nc.sync.dma_start(ka[64:65, :], alibi8_d[h : h + 1, :].bitcast(F32R))

                bcast_h = bcp.tile([128, S], FP32, tag="bc")
                # scores [q, k] -> softmax -> weights out
                for qb in range(NSB):
                    stat = qa[:, qb * 128 : (qb + 1) * 128]
                    wstrip = stripp.tile([128, S], FP32, tag="strip")
                    d4 = smallp.tile([128, NCH], FP32, tag="d4")
                    for c in range(NCH):
                        ps = mmp.tile([128, 512], FP32, tag="mm")
                        nc.tensor.matmul(
                            ps, stat, ka[:, c * 512 : (c + 1) * 512], start=True, stop=True
                        )
                        nc.scalar.activation(
                            wstrip[:, c * 512 : (c + 1) * 512],
                            ps,
                            EXPF,
                            scale=0.125,
                            accum_out=d4[:, c : c + 1],
                        )
                    rec = smallp.tile([128, 1], FP32, tag="rec")
                    nc.vector.reduce_sum(rec, d4, axis=mybir.AxisListType.X)
                    nc.vector.reciprocal(rec, rec)
                    nc.vector.tensor_scalar_mul(wstrip, wstrip, rec)
                    nc.sync.dma_start(
                        wout_d[h, qb * 128 : (qb + 1) * 128, :], wstrip
                    )
                    pr = mmp.tile([1, 128], FP32, tag="mm")
                    nc.tensor.transpose(pr, rec, ident)
                    rt = smallp.tile([1, 128], FP32, tag="rt", bufs=2)
                    nc.vector.tensor_copy(rt, pr)
                    nc.gpsimd.partition_broadcast(
                        bcast_h[:, qb * 128 : (qb + 1) * 128], rt
                    )

                # scores^T [k, q] -> exp -> context^T accumulation.
                # Stationary operand is the full 2-head V block so the PSUM
                # write covers partitions 0..127 (partition-offset PSUM writes
                # are invalid ISA); only rows r:r+64 are meaningful.
                pctx = ctxpp.tile([128, S], FP32, tag="ctx")
                for kb in range(NSB):
                    stat2 = ka[0:64, kb * 128 : (kb + 1) * 128]
                    ustrip = stripp.tile([128, S], F32R, tag="strip")
                    for c in range(NCH):
                        ps = mmp.tile([128, 512], FP32, tag="mm")
                        nc.tensor.matmul(
                            ps,
                            stat2,
                            qa[0:64, c * 512 : (c + 1) * 512],
                            start=True,
                            stop=True,
                        )
                        nc.scalar.activation(
                            ustrip[:, c * 512 : (c + 1) * 512],
                            ps,
                            EXPF,
                            scale=0.125,
                            bias=biascol_sb[:, h * NSB + kb : h * NSB + kb + 1],
                        )
                    for c in range(NCH):
                        nc.tensor.matmul(
                            pctx[:, c * 512 : (c + 1) * 512],
                            v_sb[:, kb * 256 + g * 128 : kb * 256 + (g + 1) * 128],
                            ustrip[:, c * 512 : (c + 1) * 512],
                            start=(kb == 0),
                            stop=(kb == NSB - 1),
                        )
                nc.vector.tensor_mul(
                    ctxT[g][r : r + 64, :], pctx[r : r + 64, :], bcast_h[r : r + 64, :]
                )

            # --- phase 3: output projection (partial over this core's heads) ---
            for sb_i in range(NSB):
                ost = obufp.tile([128, DM], FP32, tag="obuf")
                for half in range(2):
                    po = mmp.tile([128, 512], FP32, tag="mm")
                    for g in range(2):
                        nc.tensor.matmul(
                            po,
                            ctxT[g][:, sb_i * 128 : (sb_i + 1) * 128],
                            wo_sb[:, g * 1024 + half * 512 : g * 1024 + (half + 1) * 512],
                            start=(g == 0),
                            stop=(g == 1),
                        )
                    nc.scalar.copy(ost[:, half * 512 : (half + 1) * 512], po)
                nc.sync.dma_start(oout_d[sb_i * 128 : (sb_i + 1) * 128, :], ost)

    nc.compile()
    return nc


_NC_CACHE = []


def kernel(query, key, value, Wq, Wk, Wv, Wo):
    if not _NC_CACHE:
        _NC_CACHE.append(build_module())
    nc = _NC_CACHE[0]

    slopes = np.array(_alibi_slopes(H), dtype=np.float64)
    j = np.arange(S, dtype=np.float64)
    p = np.arange(128, dtype=np.float64)

    in_maps = []
    for c in range(NCORES):
        b, hg = c // HPC, c % HPC
        sl = slopes[hg * HPC : (hg + 1) * HPC]  # local heads
        alibi8 = np.concatenate(
            [
                (-8.0 * sl[:, None] * (2047.0 - j[None, :])).astype(np.float32),
                np.ones((1, S), np.float32),
            ],
            axis=0,
        )
        biascol = np.empty((128, HPC * NSB), np.float32)
        for hl in range(HPC):
            for kb in range(NSB):
                biascol[:, hl * NSB + kb] = (-sl[hl] * (2047.0 - (kb * 128 + p))).astype(
                    np.float32
                )
        in_maps.append(
            {
                "q": np.ascontiguousarray(query[b], np.float32),
                "k": np.ascontiguousarray(key[b], np.float32),
                "v": np.ascontiguousarray(value[b], np.float32),
                "wq": np.ascontiguousarray(Wq[:, hg * 256 : (hg + 1) * 256], np.float32),
                "wk": np.ascontiguousarray(Wk[:, hg * 256 : (hg + 1) * 256], np.float32),
                "wv": np.ascontiguousarray(Wv[:, hg * 256 : (hg + 1) * 256], np.float32),
                "wo": np.ascontiguousarray(Wo[hg * 256 : (hg + 1) * 256, :], np.float32),
                "alibi8": alibi8,
                "biascol": biascol,
            }
        )

    global _last_in_maps
    _last_in_maps = in_maps
    res = bass_utils.run_bass_kernel_spmd(nc, in_maps, core_ids=list(range(NCORES)))

    weights = np.empty((B, H, S, S), np.float32)
    output = np.zeros((B, S, DM), np.float32)
    for c in range(NCORES):
        b, hg = c // HPC, c % HPC
        weights[b, hg * HPC : (hg + 1) * HPC] = res.results[c]["wout"]
        output[b] += res.results[c]["oout"]
    return output, weights


# revision 32
# speedup vs baseline: 3.2012x; 3.2012x over previous
"""Multi-head attention (B=2, S=2048, D=1024, H=16, dk=dv=64) with ALiBi bias,
returning (output, softmax weights), on 8 Trainium2 NeuronCores.

Sharding: batch x head-group (2 x 4). Each core computes 4 heads of one batch:
QKV projections column-sharded, scores/softmax/context local, Wo row-sharded
with the cross-head partial sum reduced on the host.

Numerics: matmuls run in float32r (~13-bit mantissa, full PE rate for moving
dim >= 256). The ALiBi bias is folded into the scores matmul via an augmented
contraction row holding 8*(bias - C_h) with C_h = slope_h*2047, so the
(large) bias values only carry f32r rounding error where the softmax weight
is negligible. Softmax needs no max-reduction: exponents are <= ~6 by
construction. exp runs on ScalarE with per-partition bias + accumulated row
sums; normalization and PSUM evacuation on VectorE.

Program order is tuned for the in-order engine streams: q/k projections
first, then head 0's scores/softmax interleaved with the V projection, then
the remaining heads with their context (kblk) loop ahead of the softmax
(qblk) loop so the kernel tail is short.
"""

import math

import numpy as np

import concourse.mybir as mybir
import concourse.tile as tile
from concourse import bacc
from concourse import bass_utils

B, S, DM, H, DK, DV = 2, 2048, 1024, 16, 64, 64
HPC = 4  # heads per core
NCORES = 8
FP32 = mybir.dt.float32
F32R = mybir.dt.float32r
EXPF = mybir.ActivationFunctionType.Exp
NSB = S // 128  # 16 s-blocks
NDB = DM // 128  # 8 dmodel-blocks


def _alibi_slopes(n_heads):
    def pow2_slopes(n):
        start = 2.0 ** (-(2.0 ** (-(math.log2(n) - 3))))
        return [start * (start**i) for i in range(n)]

    if math.log2(n_heads).is_integer():
        return pow2_slopes(n_heads)
    closest = 2 ** math.floor(math.log2(n_heads))
    return pow2_slopes(closest) + _alibi_slopes(2 * closest)[0::2][: n_heads - closest]


def build_module(loop=1):
    nc = bacc.Bacc("TRN2", target_bir_lowering=False, debug=False, num_devices=NCORES)

    q_d = nc.dram_tensor("q", [S, DM], FP32, kind="ExternalInput").ap()
    k_d = nc.dram_tensor("k", [S, DM], FP32, kind="ExternalInput").ap()
    v_d = nc.dram_tensor("v", [S, DM], FP32, kind="ExternalInput").ap()
    wq_d = nc.dram_tensor("wq", [DM, HPC * DK], FP32, kind="ExternalInput").ap()
    wk_d = nc.dram_tensor("wk", [DM, HPC * DK], FP32, kind="ExternalInput").ap()
    wv_d = nc.dram_tensor("wv", [DM, HPC * DV], FP32, kind="ExternalInput").ap()
    wo_d = nc.dram_tensor("wo", [HPC * DV, DM], FP32, kind="ExternalInput").ap()
    alibi8_d = nc.dram_tensor("alibi8", [HPC + 1, S], FP32, kind="ExternalInput").ap()
    biascol_d = nc.dram_tensor("biascol", [128, HPC * NSB], FP32, kind="ExternalInput").ap()
    ident_d = nc.dram_tensor("ident", [128, 128], FP32, kind="ExternalInput").ap()
    wout_d = nc.dram_tensor("wout", [HPC, S, S], FP32, kind="ExternalOutput").ap()
    oout_d = nc.dram_tensor("oout", [2, S, DM], FP32, kind="ExternalOutput").ap()

    import contextlib

    with tile.TileContext(nc) as tc:
        with (
            tc.For_i(0, loop, 1) if loop > 1 else contextlib.nullcontext(),
            tc.tile_pool(name="persist", bufs=1) as persist,
            tc.tile_pool(name="strip", bufs=7) as stripp,
            tc.tile_pool(name="aug", bufs=3) as augp,
            tc.tile_pool(name="inp", bufs=2) as inpp,
            tc.tile_pool(name="obuf", bufs=2) as obufp,
            tc.tile_pool(name="bc", bufs=1) as bcp,
            tc.tile_pool(name="small", bufs=8) as smallp,
            tc.tile_pool(name="sc", bufs=2, space="PSUM") as scp,
            tc.tile_pool(name="ctxp", bufs=2, space="PSUM") as ctxpp,
        ):
            identr = persist.tile([128, 128], F32R, tag="identr")
            nc.sync.dma_start(identr, ident_d.bitcast(F32R))
            biascol_sb = persist.tile([128, HPC * NSB], FP32, tag="biascol")
            nc.sync.dma_start(biascol_sb, biascol_d)

            v_sb = persist.tile([128, NSB * HPC * DV], F32R, tag="vsb")
            gk = {}  # (tensor_idx, grp) -> [128, S] f32r projection outputs
            ctxT = [
                persist.tile([128, S], F32R, tag=f"ctxT{g}", name=f"ctxT{g}")
                for g in range(2)
            ]
            bcast = {}
            pctx = {}

            # --- weight shards -> SBUF (bitcast: PE consumes unrounded f32r) ---
            def load_w(wdram, nblk, wname):
                wsb = persist.tile(
                    [128, nblk * wdram.shape[1]], F32R, tag=wname, name=wname
                )
                nc.sync.dma_start(
                    wsb.rearrange("p (nb m) -> p nb m", nb=nblk),
                    wdram.rearrange("(nb p) m -> p nb m", p=128).bitcast(F32R),
                )
                return wsb

            wq_sb = load_w(wq_d, NDB, "wqsb")  # [128, 8*256]
            wk_sb = load_w(wk_d, NDB, "wksb")
            wv_sb = load_w(wv_d, NDB, "wvsb")
            wo_sb = load_w(wo_d, 2, "wosb")  # [128, 2*1024]

            # --- phase 1 emitter ---
            def proj_chunk(t_idx, src_d, wsb, chunk, spread_copies, per_j=None):
                """Transpose one 512-wide s-chunk of an input and project it."""
                is_v = t_idx == 2
                colA = stripp.tile([128, 2048], F32R, tag="strip", name="colA")
                colB = stripp.tile([128, 2048], F32R, tag="strip", name="colB")

                def colview(t, j):  # [128, 4, 128] view at s-offset j*128
                    return t.rearrange("p (c x) -> p c x", c=4)[
                        :, :, j * 128 : (j + 1) * 128
                    ]

                def col(db, lo, sz):
                    t = colA if db < 4 else colB
                    return t[:, (db % 4) * 512 + lo : (db % 4) * 512 + lo + sz]

                for j in range(4):
                    sblk = chunk * 4 + j
                    it = inpp.tile([128, DM], F32R, tag="inp")
                    nc.sync.dma_start(
                        it, src_d[sblk * 128 : (sblk + 1) * 128, :].bitcast(F32R)
                    )
                    pt = scp.tile([128, 1024], F32R, tag="sc")
                    for db in range(NDB):
                        nc.tensor.transpose(
                            pt[:, db * 128 : (db + 1) * 128],
                            it[:, db * 128 : (db + 1) * 128],
                            identr,
                        )
                    if per_j is not None:
                        per_j(chunk * 4 + j)
                    # batched PSUM evacuation, spread to ACT while it is idle
                    if spread_copies and j % 2 == 1:
                        nc.scalar.copy(
                            colview(colA, j), pt[:, 0:512].rearrange("p (c x) -> p c x", c=4)
                        )
                        nc.scalar.copy(
                            colview(colB, j), pt[:, 512:1024].rearrange("p (c x) -> p c x", c=4)
                        )
                    else:
                        nc.vector.tensor_copy(
                            colview(colA, j), pt[:, 0:512].rearrange("p (c x) -> p c x", c=4)
                        )
                        nc.vector.tensor_copy(
                            colview(colB, j), pt[:, 512:1024].rearrange("p (c x) -> p c x", c=4)
                        )
                if not is_v:
                    for g in range(2):
                        ps = ctxpp.tile([128, 1024], FP32, tag="ctx")
                        for db in range(NDB):
                            nc.tensor.matmul(
                                ps[:, 0:512],
                                wsb[:, db * 256 + g * 128 : db * 256 + (g + 1) * 128],
                                col(db, 0, 512),
                                start=(db == 0),
                                stop=(db == NDB - 1),
                            )
                        nc.vector.tensor_copy(
                            gk[(t_idx, g)][:, chunk * 512 : (chunk + 1) * 512],
                            ps[:, 0:512],
                        )
                else:
                    for j in range(4):
                        sblk = chunk * 4 + j
                        ps = ctxpp.tile([128, 1024], FP32, tag="ctx")
                        for db in range(NDB):
                            nc.tensor.matmul(
                                ps[:, 0 : HPC * DV],
                                col(db, j * 128, 128),
                                wsb[:, db * 256 : (db + 1) * 256],
                                start=(db == 0),
                                stop=(db == NDB - 1),
                            )
                        nc.vector.tensor_copy(
                            v_sb[:, sblk * 256 : (sblk + 1) * 256],
                            ps[:, 0 : HPC * DV],
                        )

            # --- phase 2 emitters ---
            def emit_aug(h):
                g, r = h // 2, (h % 2) * 64
                qa = augp.tile([65, S], F32R, tag="aug", name=f"qa{h}")
                ka = augp.tile([65, S], F32R, tag="aug", name=f"ka{h}")
                for c in range(4):
                    cs = slice(c * 512, (c + 1) * 512)
                    nc.sync.dma_start(qa[0:64, cs], gk[(0, g)][r : r + 64, cs])
                    nc.sync.dma_start(
                        qa[64:65, cs], alibi8_d[HPC : HPC + 1, cs].bitcast(F32R)
                    )
                    nc.sync.dma_start(ka[0:64, cs], gk[(1, g)][r : r + 64, cs])
                    nc.sync.dma_start(
                        ka[64:65, cs], alibi8_d[h : h + 1, cs].bitcast(F32R)
                    )
                bcast[h] = bcp.tile([128, S], FP32, tag="bc", name=f"bc{h}")
                return qa, ka

            recs = {}

            def emit_qblk(h, qa, ka, qbs, post_qb=None):
                """scores [q,k] -> softmax -> weights out, for q-blocks qbs."""
                if h not in recs:
                    recs[h] = smallp.tile(
                        [128, NSB], FP32, tag="recs", name=f"recs{h}", bufs=2
                    )
                for qb in qbs:
                    stat = qa[:, qb * 128 : (qb + 1) * 128]
                    wstrip = stripp.tile([128, S], FP32, tag="strip", name="wstrip")
                    d2 = smallp.tile([128, 2], FP32, tag="d2")
                    for half in range(2):
                        ps = scp.tile([128, 1024], FP32, tag="sc")
                        for kc in range(2):
                            lo = half * 1024 + kc * 512
                            nc.tensor.matmul(
                                ps[:, kc * 512 : (kc + 1) * 512],
                                stat,
                                ka[:, lo : lo + 512],
                                start=True,
                                stop=True,
                            )
                        nc.scalar.activation(
                            wstrip[:, half * 1024 : (half + 1) * 1024],
                            ps,
                            EXPF,
                            scale=0.125,
                            accum_out=d2[:, half : half + 1],
                        )
                    rec = recs[h][:, qb : qb + 1]
                    nc.vector.reduce_sum(rec, d2, axis=mybir.AxisListType.X)
                    nc.vector.reciprocal_approx_fast(rec, rec)
                    nc.vector.tensor_scalar_mul(wstrip, wstrip, rec)
                    nc.sync.dma_start(wout_d[h, qb * 128 : (qb + 1) * 128, :], wstrip)
                    if post_qb is not None:
                        post_qb(qb)

            def emit_bcast(h):
                """Deferred recip transposes -> bcast rows (off critical path;
                partition_broadcast needs its source at partition 0)."""
                for qb in range(NSB):
                    pr = scp.tile([128, 1024], FP32, tag="sc")
                    nc.tensor.transpose(
                        pr[0:1, 0:128], recs[h][:, qb : qb + 1], identr.bitcast(FP32)
                    )
                    rt = smallp.tile([1, 128], FP32, tag="rt", bufs=2)
                    nc.vector.tensor_copy(rt, pr[0:1, 0:128])
                    nc.gpsimd.partition_broadcast(
                        bcast[h][:, qb * 128 : (qb + 1) * 128], rt
                    )

            def emit_kblk(h, qa, ka):
                """scores^T [k,q] -> exp -> context^T accumulation.

                Stationary operand is the full 2-head V block so the PSUM
                write covers partitions 0..127; only rows r:r+64 matter."""
                g = h // 2
                pctx[h] = [
                    ctxpp.tile([128, 1024], FP32, tag="ctx", name=f"pctx{h}{i}")
                    for i in range(2)
                ]
                def ctx_mm(kb, ustrip):
                    for c in range(4):
                        nc.tensor.matmul(
                            pctx[h][c // 2][:, (c % 2) * 512 : (c % 2 + 1) * 512],
                            v_sb[:, kb * 256 + g * 128 : kb * 256 + (g + 1) * 128],
                            ustrip[:, c * 512 : (c + 1) * 512],
                            start=(kb == 0),
                            stop=(kb == NSB - 1),
                        )

                pending = []
                for kb in range(NSB):
                    stat2 = ka[0:64, kb * 128 : (kb + 1) * 128]
                    ustrip = stripp.tile([128, S], F32R, tag="strip", name="ustrip")
                    for half in range(2):
                        ps = scp.tile([128, 1024], FP32, tag="sc")
                        for kc in range(2):
                            lo = half * 1024 + kc * 512
                            nc.tensor.matmul(
                                ps[:, kc * 512 : (kc + 1) * 512],
                                stat2,
                                qa[0:64, lo : lo + 512],
                                start=True,
                                stop=True,
                            )
                        nc.scalar.activation(
                            ustrip[:, half * 1024 : (half + 1) * 1024],
                            ps,
                            EXPF,
                            scale=0.125,
                            bias=biascol_sb[:, h * NSB + kb : h * NSB + kb + 1],
                        )
                    # ctx matmuls deferred a few iterations: keeps PE from
                    # stalling on exps / freed PSUM slots before issuing the
                    # next scores
                    pending.append((kb, ustrip))
                    if len(pending) > 3:
                        ctx_mm(*pending.pop(0))
                for p in pending:
                    ctx_mm(*p)

            def emit_scale(h):
                g, r = h // 2, (h % 2) * 64
                for i in range(2):
                    nc.vector.tensor_mul(
                        ctxT[g][r : r + 64, i * 1024 : (i + 1) * 1024],
                        pctx[h][i][r : r + 64, :],
                        bcast[h][r : r + 64, i * 1024 : (i + 1) * 1024],
                    )

            # --- program order ---
            for t_idx in (0, 1):
                for g in range(2):
                    gk[(t_idx, g)] = persist.tile(
                        [128, S], F32R, tag=f"gk{t_idx}{g}", name=f"gk{t_idx}{g}"
                    )
            # alternate q/k chunks so ka's early chunks are ready sooner
            for chunk in range(4):
                proj_chunk(0, q_d, wq_sb, chunk, spread_copies=True)
                proj_chunk(1, k_d, wk_sb, chunk, spread_copies=True)

            qa0, ka0 = emit_aug(0)
            # head 0 softmax interleaved per s-block with the V projection
            for chunk in range(4):
                proj_chunk(
                    2, v_d, wv_sb, chunk, spread_copies=False,
                    per_j=lambda qb: emit_qblk(0, qa0, ka0, [qb]),
                )
            emit_kblk(0, qa0, ka0)
            emit_bcast(0)
            emit_scale(0)

            def emit_outproj_one(g, sb_i):
                # partial output projection for one ctxT group, one s-block
                po = scp.tile([128, 1024], FP32, tag="sc")
                for half in range(2):
                    nc.tensor.matmul(
                        po[:, half * 512 : (half + 1) * 512],
                        ctxT[g][:, sb_i * 128 : (sb_i + 1) * 128],
                        wo_sb[:, g * 1024 + half * 512 : g * 1024 + (half + 1) * 512],
                        start=True,
                        stop=True,
                    )
                ost = obufp.tile([128, DM], FP32, tag="obuf")
                if sb_i % 2 == 0:
                    nc.vector.tensor_copy(ost, po)
                else:
                    nc.scalar.copy(ost, po)
                nc.sync.dma_start(oout_d[g, sb_i * 128 : (sb_i + 1) * 128, :], ost)

            for h in range(1, HPC):
                qa, ka = emit_aug(h)
                emit_kblk(h, qa, ka)
                # group 0's output projection rides along head 2's softmax
                post = (lambda qb: emit_outproj_one(0, qb)) if h == 2 else None
                emit_qblk(h, qa, ka, range(NSB), post_qb=post)
                emit_bcast(h)
                emit_scale(h)
            for sb_i in range(NSB):
                emit_outproj_one(1, sb_i)

    nc.compile()
    return nc


_NC_CACHE = []
_last_in_maps = None


def kernel(query, key, value, Wq, Wk, Wv, Wo):
    if not _NC_CACHE:
        _NC_CACHE.append(build_module())
    nc = _NC_CACHE[0]

    slopes = np.array(_alibi_slopes(H), dtype=np.float64)
    j = np.arange(S, dtype=np.float64)
    p = np.arange(128, dtype=np.float64)

    in_maps = []
    for c in range(NCORES):
        b, hg = c // HPC, c % HPC
        sl = slopes[hg * HPC : (hg + 1) * HPC]  # local heads
        alibi8 = np.concatenate(
            [
                (-8.0 * sl[:, None] * (2047.0 - j[None, :])).astype(np.float32),
                np.ones((1, S), np.float32),
            ],
            axis=0,
        )
        biascol = np.empty((128, HPC * NSB), np.float32)
        for hl in range(HPC):
            for kb in range(NSB):
                biascol[:, hl * NSB + kb] = (-sl[hl] * (2047.0 - (kb * 128 + p))).astype(
                    np.float32
                )
        in_maps.append(
            {
                "q": np.ascontiguousarray(query[b], np.float32),
                "k": np.ascontiguousarray(key[b], np.float32),
                "v": np.ascontiguousarray(value[b], np.float32),
                "wq": np.ascontiguousarray(Wq[:, hg * 256 : (hg + 1) * 256], np.float32),
                "wk": np.ascontiguousarray(Wk[:, hg * 256 : (hg + 1) * 256], np.float32),
                "wv": np.ascontiguousarray(Wv[:, hg * 256 : (hg + 1) * 256], np.float32),
                "wo": np.ascontiguousarray(Wo[hg * 256 : (hg + 1) * 256, :], np.float32),
                "alibi8": alibi8,
                "biascol": biascol,
                "ident": np.eye(128, dtype=np.float32),
            }
        )

    global _last_in_maps
    _last_in_maps = in_maps
    res = bass_utils.run_bass_kernel_spmd(nc, in_maps, core_ids=list(range(NCORES)))

    weights = np.empty((B, H, S, S), np.float32)
    output = np.zeros((B, S, DM), np.float32)
    for c in range(NCORES):
        b, hg = c // HPC, c % HPC
        weights[b, hg * HPC : (hg + 1) * HPC] = res.results[c]["wout"]
        output[b] += res.results[c]["oout"].sum(axis=0)
    return output, weights
